# revision 38
# baseline (speedup 1.0000x reference)
from collections import deque
from contextlib import ExitStack

import numpy as np

import concourse.bass as bass
import concourse.mybir as mybir
import concourse.tile as tile
from concourse import bacc
from concourse.bass import ts
from concourse.bass_utils import run_bass_kernel_spmd

B, N, D, H = 8, 1024, 768, 12
HD = D // H
SCALE = HD ** -0.5
KC = D // 128
NT = N // 128
NC2 = N // 512

F32 = mybir.dt.float32
F32R = mybir.dt.float32r
F16 = mybir.dt.float16
AF = mybir.ActivationFunctionType

RATIOS = {0: 6.0, 1: 2.5, 2: 1.8, 3: 1.8, 4: 1.8, 5: 1.8, 6: 1.8,
          7: 1.8, 8: 1.8, 9: 2.5, 10: 2.5, 11: 2.5}

_CACHE: dict = {}
DEBUG = False
JUNK0 = 12


def _run(gen):
    for _ in gen:
        pass


def _emit(tc, repeat=1):
    nc = tc.nc
    xt_d = nc.dram_tensor("xt", [D, N], F32R, kind="ExternalInput").ap()
    w_d = nc.dram_tensor("w", [1, N], F32, kind="ExternalInput").ap()
    wqk_d = nc.dram_tensor("wqk", [D, 2 * D], F32R, kind="ExternalInput").ap()
    wv_d = nc.dram_tensor("wv", [D, D], F32R, kind="ExternalInput").ap()
    wm_d = nc.dram_tensor("wmsa", [D, D], F16, kind="ExternalInput").ap()
    bm_d = nc.dram_tensor("bmsa", [D], F32, kind="ExternalInput").ap()
    y_d = nc.dram_tensor("yt", [D, N], F16, kind="ExternalOutput").ap()

    for _rep in range(repeat):
        _emit_body(tc, xt_d, w_d, wqk_d, wv_d, wm_d, bm_d, y_d)


def _emit_body(tc, xt_d, w_d, wqk_d, wv_d, wm_d, bm_d, y_d):
    nc = tc.nc
    with ExitStack() as s1:
        const = s1.enter_context(tc.tile_pool(name="const", bufs=1))
        pxt = s1.enter_context(tc.tile_pool(name="pxt", bufs=1))
        pwqs = s1.enter_context(tc.tile_pool(name="pwqs", bufs=2))
        pqk = s1.enter_context(tc.tile_pool(name="pqk", bufs=1))
        pwv = s1.enter_context(tc.tile_pool(name="pwv", bufs=1))
        pwm = s1.enter_context(tc.tile_pool(name="pwm", bufs=1))
        pv = s1.enter_context(tc.tile_pool(name="pv", bufs=1))
        pe_ = s1.enter_context(tc.tile_pool(name="pe", bufs=2 * NT + 1))
        pob = s1.enter_context(tc.tile_pool(name="pob", bufs=2))
        pot = s1.enter_context(tc.tile_pool(name="pot", bufs=1))
        prc = s1.enter_context(tc.tile_pool(name="prc", bufs=2))
        pfin = s1.enter_context(tc.tile_pool(name="pfin", bufs=1))
        pdn = s1.enter_context(tc.tile_pool(name="pdn", bufs=1))
        psS = s1.enter_context(tc.tile_pool(name="psS", bufs=2, space="PSUM"))
        psO = s1.enter_context(tc.tile_pool(name="psO", bufs=1, space="PSUM"))
        psF = s1.enter_context(tc.tile_pool(name="psF", bufs=2, space="PSUM"))

        wqk3 = wqk_d.rearrange("(c p) e -> p c e", p=128)

        def load_wq(m, eng=nc.sync):
            t = pwqs.tile([128, KC * 128], F32R, tag="wqs", name="wqs")
            eng.dma_start(
                t[:].rearrange("p (c e) -> p c e", e=128),
                wqk3[:, :, ts(m, 128)],
            )
            return t

        wq_pre = load_wq(0, eng=nc.scalar)
        wq_pre2 = load_wq(KC, eng=nc.scalar)
        xtt = [pxt.tile([128, N], F32R, tag=f"xt{c}", name=f"xt{c}") for c in range(KC)]
        for c in range(KC):
            eng = nc.sync if c % 2 == 0 else nc.scalar
            eng.dma_start(xtt[c][:], xt_d[ts(c, 128), :])
        w_row = pdn.tile([1, N], F32, tag="w_row", name="w_row")
        nc.sync.dma_start(w_row[:], w_d[:])
        wcol = const.tile([128, NT], F32, tag="wcol")
        nc.sync.dma_start(wcol[:], w_d[0, :].rearrange("(r p) -> p r", p=128))

        onescol = const.tile([128, H], F16, tag="onescol")
        nc.vector.memset(onescol[:], 1.0)
        junk = const.tile([128, 512], F16, tag="junk")
        nc.vector.memset(junk[:], 1.0)
        idn = const.tile([128, 128], F16, tag="idn")
        nc.gpsimd.affine_select(
            idn[:], junk[:, 0:128], pattern=[[-1, 128]],
            compare_op=mybir.AluOpType.is_equal, fill=0.0,
            channel_multiplier=1, base=0,
        )
        wb = const.tile([128, N], F32, tag="wb")
        nc.gpsimd.partition_broadcast(wb[:], w_row[:])

        wvt = [pwv.tile([128, D], F32R, tag=f"wv{c}", name=f"wv{c}") for c in range(KC)]
        for c in range(KC):
            eng = nc.sync if c % 2 == 0 else nc.scalar
            eng.dma_start(wvt[c][:], wv_d[ts(c, 128), :])
        bias = const.tile([128, KC], F32, tag="bias")
        nc.sync.dma_start(bias[:], bm_d.rearrange("(c p) -> p c", p=128))
        wmt = [pwm.tile([128, D], F16, tag=f"wm{c}", name=f"wm{c}") for c in range(KC)]
        for c in range(KC):
            eng = nc.sync if c % 2 == 0 else nc.scalar
            eng.dma_start(wmt[c][:], wm_d[ts(c, 128), :])

        qkt = [pqk.tile([128, N], F32R, tag=f"qk{m}", name=f"qk{m}")
               for m in range(2 * KC)]
        vt = [pv.tile([128, H * (HD + 1)], F16, tag=f"v{r}", name=f"v{r}")
              for r in range(NT)]
        ott = [pot.tile([128, N], F16, tag=f"ot{c}", name=f"ot{c}") for c in range(KC)]
        fin = [pfin.tile([128, N], F16, tag=f"fin{c}", name=f"fin{c}")
               for c in range(KC)]
        OFF = [0, 65, 130, 195, 512, 577, 642, 707]
        obufs: dict = {}
        if DEBUG:
            e_snap = const.tile([128, N], F32, tag="esnap")
            ob_snap = const.tile([128, 128], F32, tag="obsnap")

        def junk_mm(n):
            for _ in range(n):
                ps = psF.tile([128, 512], F32, tag="psF", name="psj")
                nc.tensor.matmul(ps[0:1, :], onescol[:, 0:1], junk[:],
                                 start=True, stop=True)

        junk_mm(JUNK0)

        ps_qk0 = psS.tile([128, N], F32, tag="psS", name="psq0")
        ps_qk6 = psS.tile([128, N], F32, tag="psS", name="psq6")
        for c in range(KC):
            for j in range(NC2):
                nc.tensor.matmul(ps_qk0[:, ts(j, 512)], wq_pre[:, ts(c, 128)],
                                 xtt[c][:, ts(j, 512)],
                                 start=(c == 0), stop=(c == KC - 1))
            for j in range(NC2):
                nc.tensor.matmul(ps_qk6[:, ts(j, 512)], wq_pre2[:, ts(c, 128)],
                                 xtt[c][:, ts(j, 512)],
                                 start=(c == 0), stop=(c == KC - 1))
            if c < KC - 1:
                junk_mm(3)
        nc.vector.tensor_mul(qkt[0][:], ps_qk0[:], wb[:])
        nc.vector.tensor_mul(qkt[KC][:], ps_qk6[:], wb[:])

        def veng(i):
            return nc.vector

        def gen_v():
            for r in range(NT):
                v3 = vt[r][:].rearrange("p (h e) -> p h e", e=HD + 1)
                nc.vector.tensor_copy(
                    v3[:, :, HD : HD + 1],
                    onescol[:].rearrange("p (h o) -> p h o", o=1),
                )
                for pi, (off, wd) in enumerate(((0, 512), (512, 256))):
                    ps = psF.tile([128, wd], F32, tag="psF", name="psv")
                    for c in range(KC):
                        nc.tensor.matmul(ps[:], xtt[c][:, ts(r, 128)],
                                         wvt[c][:, off : off + wd],
                                         start=(c == 0), stop=(c == KC - 1))
                        yield
                    h0, h1 = (0, 8) if pi == 0 else (8, 12)
                    nc.vector.tensor_scalar_mul(
                        v3[:, h0:h1, 0:HD],
                        ps[:].rearrange("p (h e) -> p h e", e=HD),
                        wcol[:, r : r + 1],
                    )

        def gen_qk(m):
            wq_m = load_wq(m, eng=nc.scalar if m % 2 else nc.sync)
            for j in range(NC2):
                ps = psF.tile([128, 512], F32, tag="psF", name="psq")
                for c in range(KC):
                    nc.tensor.matmul(ps[:], wq_m[:, ts(c, 128)],
                                     xtt[c][:, ts(j, 512)],
                                     start=(c == 0), stop=(c == KC - 1))
                    yield
                nc.vector.tensor_mul(
                    qkt[m][:, ts(j, 512)], ps[:], wb[:, ts(j, 512)]
                )

        def gen_final(ks, borrow=False):
            first = ks[0] == 0
            for cp in range(KC):
                for j in range(NC2):
                    pool = psS if (borrow and (cp * 2 + j) % 2) else psF
                    tag = "psS" if pool is psS else "psF"
                    ps = pool.tile([128, 512], F32, tag=tag, name="psy")
                    for i, k in enumerate(ks):
                        nc.tensor.matmul(ps[:], wmt[k][:, ts(cp, 128)],
                                         ott[k][:, ts(j, 512)],
                                         start=(i == 0), stop=(i == len(ks) - 1))
                        yield
                    dst = fin[cp][:, ts(j, 512)]
                    eng = veng(cp * 2 + j)
                    if first:
                        eng.tensor_scalar_add(dst, ps[:], bias[:, cp : cp + 1])
                    else:
                        eng.tensor_add(dst, ps[:], dst)

        fillq: deque = deque()

        def pump(units):
            while units >= 1.0 and fillq:
                try:
                    next(fillq[0])
                    units -= 1.0
                except StopIteration:
                    fillq.popleft()

        e_store: dict = {}

        def gen_S(h):
            c, half = h // 2, h % 2
            qt, kt = qkt[c], qkt[KC + c]
            qr = HD * half
            es = []
            for r in range(NT):
                ps = psS.tile([128, N], F32, tag="psS", name="ps")
                for j in range(NC2):
                    nc.tensor.matmul(ps[:, ts(j, 512)], kt[qr : qr + HD, ts(r, 128)],
                                     qt[qr : qr + HD, ts(j, 512)],
                                     start=True, stop=True)
                e = pe_.tile([128, N], F16, tag="e", name="e")
                nc.scalar.activation(e[:], ps[:], AF.Exp, scale=SCALE)
                if DEBUG and h == 0 and r == 0:
                    nc.vector.tensor_copy(e_snap[:], e[:])
                es.append(e)
                yield
            e_store[h] = es

        def gen_O(h):
            c, half = h // 2, h % 2
            es = e_store.pop(h)
            po = psO.tile([128, 1024], F32, tag="po", name="po")
            for qc in range(NT):
                for r in range(NT):
                    nc.tensor.matmul(po[:, OFF[qc] : OFF[qc] + HD + 1],
                                     es[r][:, ts(qc, 128)],
                                     vt[r][:, h * (HD + 1) : (h + 1) * (HD + 1)],
                                     start=(r == 0), stop=(r == NT - 1))
                yield
            rcs = prc.tile([128, NT], F32, tag="rcs", name="rcs")
            for half_b in range(2):
                nc.vector.tensor_copy(
                    rcs[:, 4 * half_b : 4 * half_b + 4].rearrange(
                        "p (g o) -> p g o", o=1),
                    po[:, 512 * half_b + HD : 512 * half_b + HD + 4 * (HD + 1)]
                    .rearrange("p (g e) -> p g e", e=HD + 1)[:, :, 0:1],
                )
            rc = prc.tile([128, NT], F32, tag="rc", name="rc")
            nc.vector.reciprocal(rc[:], rcs[:])
            for qc in range(NT):
                if half == 0:
                    obufs[(c, qc)] = pob.tile([128, 128], F16, tag=f"ob{qc}",
                                              name=f"ob{qc}")
                ob = obufs[(c, qc)]
                nc.vector.tensor_scalar_mul(
                    ob[:, HD * half : HD * half + HD],
                    po[:, OFF[qc] : OFF[qc] + HD],
                    rc[:, qc : qc + 1],
                )
                if DEBUG and h == 1 and qc == 0:
                    nc.vector.tensor_copy(ob_snap[:], ob[:])
            yield

        def gen_T(c, use_act=False):
            for qc in range(NT):
                pt = psF.tile([128, 128], F16, tag="psF", name="pt")
                nc.tensor.transpose(pt[:], obufs.pop((c, qc))[:], idn[:])
                if use_act and qc % 2 == 1:
                    nc.scalar.activation(ott[c][:, ts(qc, 128)], pt[:],
                                         AF.Identity)
                else:
                    nc.vector.tensor_copy(ott[c][:, ts(qc, 128)], pt[:])
                if qc % 4 == 3:
                    yield

        v_gen = gen_v()
        fillq.append(v_gen)
        qk_gens = {}
        for m in (1, KC + 1, 2, KC + 2, 3, KC + 3, 4, KC + 4, 5, KC + 5):
            g = gen_qk(m)
            qk_gens[m] = g
            fillq.append(g)

        def drain(*targets):
            while any(g in fillq for g in targets):
                pump(1.0)

        t_gens = {}
        prev_o = None
        for h in range(H):
            if h == 1:
                drain(v_gen)
            if h >= 2 and h % 2 == 0:
                drain(qk_gens[h // 2], qk_gens[KC + h // 2])
            if h == 9:
                drain(*(t_gens[c] for c in range(4)))
                fillq.append(gen_final((0, 1, 2, 3)))
            elif h == 11:
                drain(t_gens[4])
                fillq.append(gen_final((4,)))
            ratio = RATIOS[h]
            for _ in gen_S(h):
                if prev_o is not None:
                    next(prev_o, None)
                pump(ratio)
            if prev_o is not None:
                _run(prev_o)
                if h % 2 == 0 and h >= 2:
                    t_gens[(h - 1) // 2] = gen_T((h - 1) // 2)
                    fillq.append(t_gens[(h - 1) // 2])
            prev_o = gen_O(h)
        _run(prev_o)
        t_gens[5] = gen_T(5, use_act=True)
        fillq.append(t_gens[5])
        drain(t_gens[5])
        fillq.append(gen_final((5,), borrow=True))
        while fillq:
            pump(1e9)

        for cp in range(KC):
            eng = nc.sync if cp % 2 == 0 else nc.scalar
            eng.dma_start(y_d[ts(cp, 128), :], fin[cp][:])


def _build(repeat=1):
    key = ("nc", repeat)
    if key not in _CACHE:
        nc = bacc.Bacc("TRN2", target_bir_lowering=False, debug=False, num_devices=B)
        with tile.TileContext(nc) as tc:
            _emit(tc, repeat=repeat)
        nc.compile()
        _CACHE[key] = nc
    return _CACHE[key]


def kernel(x, weight, W_qkv, W_msa, b_msa):
    nc = _build()
    x = np.asarray(x, dtype=np.float32)
    weight = np.asarray(weight, dtype=np.float32)
    W_qkv = np.asarray(W_qkv, dtype=np.float32)
    wqk = np.ascontiguousarray(W_qkv[:, : 2 * D])
    wv = np.ascontiguousarray(W_qkv[:, 2 * D :])
    wm16 = np.asarray(W_msa, dtype=np.float16)
    in_maps = []
    for b in range(B):
        in_maps.append(
            {
                "xt": np.ascontiguousarray(x[b].T),
                "w": np.ascontiguousarray(weight[b : b + 1]),
                "wqk": wqk,
                "wv": wv,
                "wmsa": wm16,
                "bmsa": np.asarray(b_msa, dtype=np.float32),
            }
        )
    res = run_bass_kernel_spmd(nc, in_maps, list(range(B)))
    out = np.stack([res.results[b]["yt"].T for b in range(B)], axis=0)
    return np.ascontiguousarray(out.astype(np.float32))


# revision 39
# speedup vs baseline: 1.0179x; 1.0179x over previous
from collections import deque
from contextlib import ExitStack

import numpy as np

import concourse.bass as bass
import concourse.mybir as mybir
import concourse.tile as tile
from concourse import bacc
from concourse.bass import ts
from concourse.bass_utils import run_bass_kernel_spmd

B, N, D, H = 8, 1024, 768, 12
HD = D // H
SCALE = HD ** -0.5
KC = D // 128
NT = N // 128
NC2 = N // 512

F32 = mybir.dt.float32
F32R = mybir.dt.float32r
F16 = mybir.dt.float16
AF = mybir.ActivationFunctionType

RATIOS = {0: 6.0, 1: 2.5, 2: 1.8, 3: 1.8, 4: 1.8, 5: 1.8, 6: 1.8,
          7: 1.8, 8: 1.8, 9: 2.5, 10: 2.5, 11: 2.5}

_CACHE: dict = {}
DEBUG = False
JUNK0 = 12


def _run(gen):
    for _ in gen:
        pass


def _emit(tc, repeat=1):
    nc = tc.nc
    xt_d = nc.dram_tensor("xt", [D, N], F32R, kind="ExternalInput").ap()
    w_d = nc.dram_tensor("w", [1, N], F32, kind="ExternalInput").ap()
    wqk_d = nc.dram_tensor("wqk", [D, 2 * D], F32R, kind="ExternalInput").ap()
    wv_d = nc.dram_tensor("wv", [D, D], F32R, kind="ExternalInput").ap()
    wm_d = nc.dram_tensor("wmsa", [D, D], F16, kind="ExternalInput").ap()
    bm_d = nc.dram_tensor("bmsa", [D], F32, kind="ExternalInput").ap()
    y_d = nc.dram_tensor("yt", [D, N], F16, kind="ExternalOutput").ap()

    for _rep in range(repeat):
        _emit_body(tc, xt_d, w_d, wqk_d, wv_d, wm_d, bm_d, y_d)


def _emit_body(tc, xt_d, w_d, wqk_d, wv_d, wm_d, bm_d, y_d):
    nc = tc.nc
    with ExitStack() as s1:
        const = s1.enter_context(tc.tile_pool(name="const", bufs=1))
        pxt = s1.enter_context(tc.tile_pool(name="pxt", bufs=1))
        pwqs = s1.enter_context(tc.tile_pool(name="pwqs", bufs=2))
        pqk = s1.enter_context(tc.tile_pool(name="pqk", bufs=1))
        pwv = s1.enter_context(tc.tile_pool(name="pwv", bufs=1))
        pwm = s1.enter_context(tc.tile_pool(name="pwm", bufs=1))
        pv = s1.enter_context(tc.tile_pool(name="pv", bufs=1))
        pe_ = s1.enter_context(tc.tile_pool(name="pe", bufs=2 * NT + 1))
        pob = s1.enter_context(tc.tile_pool(name="pob", bufs=2))
        pot = s1.enter_context(tc.tile_pool(name="pot", bufs=1))
        prc = s1.enter_context(tc.tile_pool(name="prc", bufs=2))
        pfin = s1.enter_context(tc.tile_pool(name="pfin", bufs=1))
        pdn = s1.enter_context(tc.tile_pool(name="pdn", bufs=1))
        psS = s1.enter_context(tc.tile_pool(name="psS", bufs=2, space="PSUM"))
        psO = s1.enter_context(tc.tile_pool(name="psO", bufs=1, space="PSUM"))
        psF = s1.enter_context(tc.tile_pool(name="psF", bufs=2, space="PSUM"))

        wqk3 = wqk_d.rearrange("(c p) e -> p c e", p=128)

        def load_wq(m, eng=nc.sync):
            t = pwqs.tile([128, KC * 128], F32R, tag="wqs", name="wqs")
            eng.dma_start(
                t[:].rearrange("p (c e) -> p c e", e=128),
                wqk3[:, :, ts(m, 128)],
            )
            return t

        wq_pre = load_wq(0, eng=nc.scalar)
        wq_pre2 = load_wq(KC, eng=nc.scalar)
        xtt = [pxt.tile([128, N], F32R, tag=f"xt{c}", name=f"xt{c}") for c in range(KC)]
        for c in range(KC):
            eng = nc.sync if c % 2 == 0 else nc.scalar
            eng.dma_start(xtt[c][:], xt_d[ts(c, 128), :])
        w_row = pdn.tile([1, N], F32, tag="w_row", name="w_row")
        nc.sync.dma_start(w_row[:], w_d[:])
        wcol = const.tile([128, NT], F32, tag="wcol")
        nc.sync.dma_start(wcol[:], w_d[0, :].rearrange("(r p) -> p r", p=128))

        onescol = const.tile([128, H], F16, tag="onescol")
        nc.vector.memset(onescol[:], 1.0)
        junk = const.tile([128, 512], F16, tag="junk")
        nc.vector.memset(junk[:], 1.0)
        idn = const.tile([128, 128], F16, tag="idn")
        nc.gpsimd.affine_select(
            idn[:], junk[:, 0:128], pattern=[[-1, 128]],
            compare_op=mybir.AluOpType.is_equal, fill=0.0,
            channel_multiplier=1, base=0,
        )
        wb = const.tile([128, N], F32, tag="wb")
        nc.gpsimd.partition_broadcast(wb[:], w_row[:])

        wvt = [pwv.tile([128, D], F32R, tag=f"wv{c}", name=f"wv{c}") for c in range(KC)]
        for c in range(KC):
            eng = nc.sync if c % 2 == 0 else nc.scalar
            eng.dma_start(wvt[c][:], wv_d[ts(c, 128), :])
        bias = const.tile([128, KC], F32, tag="bias")
        nc.sync.dma_start(bias[:], bm_d.rearrange("(c p) -> p c", p=128))
        wmt = [pwm.tile([128, D], F16, tag=f"wm{c}", name=f"wm{c}") for c in range(KC)]
        for c in range(KC):
            eng = nc.sync if c % 2 == 0 else nc.scalar
            eng.dma_start(wmt[c][:], wm_d[ts(c, 128), :])

        qkt = [pqk.tile([128, N], F32R, tag=f"qk{m}", name=f"qk{m}")
               for m in range(2 * KC)]
        vt = [pv.tile([128, H * (HD + 1)], F16, tag=f"v{r}", name=f"v{r}")
              for r in range(NT)]
        ott = [pot.tile([128, N], F16, tag=f"ot{c}", name=f"ot{c}") for c in range(KC)]
        fin = [pfin.tile([128, N], F16, tag=f"fin{c}", name=f"fin{c}")
               for c in range(KC)]
        OFF = [0, 65, 130, 195, 512, 577, 642, 707]
        obufs: dict = {}
        if DEBUG:
            e_snap = const.tile([128, N], F32, tag="esnap")
            ob_snap = const.tile([128, 128], F32, tag="obsnap")

        def junk_mm(n):
            for _ in range(n):
                ps = psF.tile([128, 512], F32, tag="psF", name="psj")
                nc.tensor.matmul(ps[0:1, :], onescol[:, 0:1], junk[:],
                                 start=True, stop=True)

        junk_mm(JUNK0)

        ps_qk0 = psS.tile([128, N], F32, tag="psS", name="psq0")
        ps_qk6 = psS.tile([128, N], F32, tag="psS", name="psq6")
        for c in range(KC):
            for j in range(NC2):
                nc.tensor.matmul(ps_qk0[:, ts(j, 512)], wq_pre[:, ts(c, 128)],
                                 xtt[c][:, ts(j, 512)],
                                 start=(c == 0), stop=(c == KC - 1))
            for j in range(NC2):
                nc.tensor.matmul(ps_qk6[:, ts(j, 512)], wq_pre2[:, ts(c, 128)],
                                 xtt[c][:, ts(j, 512)],
                                 start=(c == 0), stop=(c == KC - 1))
            if c < KC - 1:
                junk_mm(3)
        nc.vector.tensor_mul(qkt[0][:], ps_qk0[:], wb[:])
        nc.vector.tensor_mul(qkt[KC][:], ps_qk6[:], wb[:])

        def veng(i):
            return nc.vector

        def gen_v():
            for r in range(NT):
                v3 = vt[r][:].rearrange("p (h e) -> p h e", e=HD + 1)
                nc.vector.tensor_copy(
                    v3[:, :, HD : HD + 1],
                    onescol[:].rearrange("p (h o) -> p h o", o=1),
                )
                for pi, (off, wd) in enumerate(((0, 512), (512, 256))):
                    ps = psF.tile([128, wd], F32, tag="psF", name="psv")
                    for c in range(KC):
                        nc.tensor.matmul(ps[:], xtt[c][:, ts(r, 128)],
                                         wvt[c][:, off : off + wd],
                                         start=(c == 0), stop=(c == KC - 1))
                        yield
                    h0, h1 = (0, 8) if pi == 0 else (8, 12)
                    nc.vector.tensor_scalar_mul(
                        v3[:, h0:h1, 0:HD],
                        ps[:].rearrange("p (h e) -> p h e", e=HD),
                        wcol[:, r : r + 1],
                    )

        def gen_qk(m):
            wq_m = load_wq(m, eng=nc.scalar if m % 2 else nc.sync)
            for j in range(NC2):
                ps = psF.tile([128, 512], F32, tag="psF", name="psq")
                for c in range(KC):
                    nc.tensor.matmul(ps[:], wq_m[:, ts(c, 128)],
                                     xtt[c][:, ts(j, 512)],
                                     start=(c == 0), stop=(c == KC - 1))
                    yield
                nc.vector.tensor_mul(
                    qkt[m][:, ts(j, 512)], ps[:], wb[:, ts(j, 512)]
                )

        def gen_final(ks, borrow=False):
            first = ks[0] == 0
            for cp in range(KC):
                for j in range(NC2):
                    pool = psS if (borrow and (cp * 2 + j) % 2) else psF
                    tag = "psS" if pool is psS else "psF"
                    ps = pool.tile([128, 512], F32, tag=tag, name="psy")
                    for i, k in enumerate(ks):
                        nc.tensor.matmul(ps[:], wmt[k][:, ts(cp, 128)],
                                         ott[k][:, ts(j, 512)],
                                         start=(i == 0), stop=(i == len(ks) - 1))
                        yield
                    dst = fin[cp][:, ts(j, 512)]
                    eng = veng(cp * 2 + j)
                    if first:
                        eng.tensor_scalar_add(dst, ps[:], bias[:, cp : cp + 1])
                    else:
                        eng.tensor_add(dst, ps[:], dst)

        fillq: deque = deque()

        def pump(units):
            while units >= 1.0 and fillq:
                try:
                    next(fillq[0])
                    units -= 1.0
                except StopIteration:
                    fillq.popleft()

        e_store: dict = {}

        def gen_S(h):
            c, half = h // 2, h % 2
            qt, kt = qkt[c], qkt[KC + c]
            qr = HD * half
            es = []
            for r in range(NT):
                ps = psS.tile([128, N], F32, tag="psS", name="ps")
                for j in range(NC2):
                    nc.tensor.matmul(ps[:, ts(j, 512)], kt[qr : qr + HD, ts(r, 128)],
                                     qt[qr : qr + HD, ts(j, 512)],
                                     start=True, stop=True)
                e = pe_.tile([128, N], F16, tag="e", name="e")
                nc.scalar.activation(e[:], ps[:], AF.Exp, scale=SCALE)
                if DEBUG and h == 0 and r == 0:
                    nc.vector.tensor_copy(e_snap[:], e[:])
                es.append(e)
                yield
            e_store[h] = es

        def gen_O(h):
            c, half = h // 2, h % 2
            es = e_store.pop(h)
            po = psO.tile([128, 1024], F32, tag="po", name="po")
            for qc in range(NT):
                for r in range(NT):
                    nc.tensor.matmul(po[:, OFF[qc] : OFF[qc] + HD + 1],
                                     es[r][:, ts(qc, 128)],
                                     vt[r][:, h * (HD + 1) : (h + 1) * (HD + 1)],
                                     start=(r == 0), stop=(r == NT - 1))
                yield
            rcs = prc.tile([128, NT], F32, tag="rcs", name="rcs")
            for half_b in range(2):
                nc.vector.tensor_copy(
                    rcs[:, 4 * half_b : 4 * half_b + 4].rearrange(
                        "p (g o) -> p g o", o=1),
                    po[:, 512 * half_b + HD : 512 * half_b + HD + 4 * (HD + 1)]
                    .rearrange("p (g e) -> p g e", e=HD + 1)[:, :, 0:1],
                )
            rc = prc.tile([128, NT], F32, tag="rc", name="rc")
            nc.vector.reciprocal(rc[:], rcs[:])
            for qc in range(NT):
                if half == 0:
                    obufs[(c, qc)] = pob.tile([128, 128], F16, tag=f"ob{qc}",
                                              name=f"ob{qc}")
                ob = obufs[(c, qc)]
                nc.vector.tensor_scalar_mul(
                    ob[:, HD * half : HD * half + HD],
                    po[:, OFF[qc] : OFF[qc] + HD],
                    rc[:, qc : qc + 1],
                )
                if DEBUG and h == 1 and qc == 0:
                    nc.vector.tensor_copy(ob_snap[:], ob[:])
                if half == 1:
                    pt = psF.tile([128, 128], F16, tag="psF", name="pt")
                    nc.tensor.transpose(pt[:], obufs.pop((c, qc))[:], idn[:])
                    if h == H - 1 and qc % 2 == 1:
                        nc.scalar.activation(ott[c][:, ts(qc, 128)], pt[:],
                                             AF.Identity)
                    else:
                        nc.vector.tensor_copy(ott[c][:, ts(qc, 128)], pt[:])
            yield

        v_gen = gen_v()
        fillq.append(v_gen)
        qk_gens = {}
        for m in (1, KC + 1, 2, KC + 2, 3, KC + 3, 4, KC + 4, 5, KC + 5):
            g = gen_qk(m)
            qk_gens[m] = g
            fillq.append(g)

        def drain(*targets):
            while any(g in fillq for g in targets):
                pump(1.0)

        prev_o = None
        for h in range(H):
            if h == 1:
                drain(v_gen)
            if h >= 2 and h % 2 == 0:
                drain(qk_gens[h // 2], qk_gens[KC + h // 2])
            if h == 8:
                fillq.append(gen_final((0, 1, 2, 3)))
            elif h == 11:
                fillq.append(gen_final((4,)))
            ratio = RATIOS[h]
            for _ in gen_S(h):
                if prev_o is not None:
                    next(prev_o, None)
                pump(ratio)
            if prev_o is not None:
                _run(prev_o)
            prev_o = gen_O(h)
        _run(prev_o)
        fillq.append(gen_final((5,), borrow=True))
        while fillq:
            pump(1e9)

        for cp in range(KC):
            eng = nc.sync if cp % 2 == 0 else nc.scalar
            eng.dma_start(y_d[ts(cp, 128), :], fin[cp][:])


def _build(repeat=1):
    key = ("nc", repeat)
    if key not in _CACHE:
        nc = bacc.Bacc("TRN2", target_bir_lowering=False, debug=False, num_devices=B)
        with tile.TileContext(nc) as tc:
            _emit(tc, repeat=repeat)
        nc.compile()
        _CACHE[key] = nc
    return _CACHE[key]


def kernel(x, weight, W_qkv, W_msa, b_msa):
    nc = _build()
    x = np.asarray(x, dtype=np.float32)
    weight = np.asarray(weight, dtype=np.float32)
    W_qkv = np.asarray(W_qkv, dtype=np.float32)
    wqk = np.ascontiguousarray(W_qkv[:, : 2 * D])
    wv = np.ascontiguousarray(W_qkv[:, 2 * D :])
    wm16 = np.asarray(W_msa, dtype=np.float16)
    in_maps = []
    for b in range(B):
        in_maps.append(
            {
                "xt": np.ascontiguousarray(x[b].T),
                "w": np.ascontiguousarray(weight[b : b + 1]),
                "wqk": wqk,
                "wv": wv,
                "wmsa": wm16,
                "bmsa": np.asarray(b_msa, dtype=np.float32),
            }
        )
    res = run_bass_kernel_spmd(nc, in_maps, list(range(B)))
    out = np.stack([res.results[b]["yt"].T for b in range(B)], axis=0)
    return np.ascontiguousarray(out.astype(np.float32))


# revision 40
# speedup vs baseline: 1.0820x; 1.0629x over previous
from collections import deque
from contextlib import ExitStack

import numpy as np

import concourse.bass as bass
import concourse.mybir as mybir
import concourse.tile as tile
from concourse import bacc
from concourse.bass import ts
from concourse.bass_utils import run_bass_kernel_spmd

B, N, D, H = 8, 1024, 768, 12
HD = D // H
SCALE = HD ** -0.5
KC = D // 128
NT = N // 128
NC2 = N // 512

F32 = mybir.dt.float32
F32R = mybir.dt.float32r
F16 = mybir.dt.float16
AF = mybir.ActivationFunctionType

RATIOS = {0: 6.0, 1: 2.5, 2: 1.8, 3: 1.8, 4: 1.8, 5: 1.8, 6: 1.8,
          7: 1.8, 8: 1.8, 9: 2.5, 10: 2.5, 11: 2.5}

_CACHE: dict = {}
DEBUG = False
JUNK0 = 12


def _run(gen):
    for _ in gen:
        pass


def _emit(tc, repeat=1):
    nc = tc.nc
    xt_d = nc.dram_tensor("xt", [D, N], F32R, kind="ExternalInput").ap()
    w_d = nc.dram_tensor("w", [1, N], F32, kind="ExternalInput").ap()
    wqk_d = nc.dram_tensor("wqk", [D, 2 * D], F32R, kind="ExternalInput").ap()
    wv_d = nc.dram_tensor("wv", [D, D], F32R, kind="ExternalInput").ap()
    wm_d = nc.dram_tensor("wmsa", [D, D], F16, kind="ExternalInput").ap()
    bm_d = nc.dram_tensor("bmsa", [D], F32, kind="ExternalInput").ap()
    y_d = nc.dram_tensor("yt", [D, N], F16, kind="ExternalOutput").ap()

    for _rep in range(repeat):
        _emit_body(tc, xt_d, w_d, wqk_d, wv_d, wm_d, bm_d, y_d)


def _emit_body(tc, xt_d, w_d, wqk_d, wv_d, wm_d, bm_d, y_d):
    nc = tc.nc
    with ExitStack() as s1:
        const = s1.enter_context(tc.tile_pool(name="const", bufs=1))
        pxt = s1.enter_context(tc.tile_pool(name="pxt", bufs=1))
        pwqs = s1.enter_context(tc.tile_pool(name="pwqs", bufs=2))
        pqk = s1.enter_context(tc.tile_pool(name="pqk", bufs=1))
        pwv = s1.enter_context(tc.tile_pool(name="pwv", bufs=1))
        pwm = s1.enter_context(tc.tile_pool(name="pwm", bufs=1))
        pv = s1.enter_context(tc.tile_pool(name="pv", bufs=1))
        pe_ = s1.enter_context(tc.tile_pool(name="pe", bufs=2 * NT + 1))
        pob = s1.enter_context(tc.tile_pool(name="pob", bufs=2))
        pot = s1.enter_context(tc.tile_pool(name="pot", bufs=1))
        prc = s1.enter_context(tc.tile_pool(name="prc", bufs=2))
        pfin = s1.enter_context(tc.tile_pool(name="pfin", bufs=1))
        pdn = s1.enter_context(tc.tile_pool(name="pdn", bufs=1))
        psS = s1.enter_context(tc.tile_pool(name="psS", bufs=2, space="PSUM"))
        psO = s1.enter_context(tc.tile_pool(name="psO", bufs=1, space="PSUM"))
        psF = s1.enter_context(tc.tile_pool(name="psF", bufs=2, space="PSUM"))

        wqk3 = wqk_d.rearrange("(c p) e -> p c e", p=128)

        def load_wq(m, eng=nc.sync):
            t = pwqs.tile([128, KC * 128], F32R, tag="wqs", name="wqs")
            eng.dma_start(
                t[:].rearrange("p (c e) -> p c e", e=128),
                wqk3[:, :, ts(m, 128)],
            )
            return t

        wq_pre = load_wq(0, eng=nc.scalar)
        wq_pre2 = load_wq(KC, eng=nc.scalar)
        xtt = [pxt.tile([128, N], F32R, tag=f"xt{c}", name=f"xt{c}") for c in range(KC)]
        for c in range(KC):
            eng = nc.sync if c % 2 == 0 else nc.scalar
            eng.dma_start(xtt[c][:], xt_d[ts(c, 128), :])
        w_row = pdn.tile([1, N], F32, tag="w_row", name="w_row")
        nc.sync.dma_start(w_row[:], w_d[:])
        wcol = const.tile([128, NT], F32, tag="wcol")
        nc.sync.dma_start(wcol[:], w_d[0, :].rearrange("(r p) -> p r", p=128))

        onescol = const.tile([128, H], F16, tag="onescol")
        nc.vector.memset(onescol[:], 1.0)
        junk = const.tile([128, 512], F16, tag="junk")
        nc.vector.memset(junk[:], 1.0)
        idn = const.tile([128, 128], F16, tag="idn")
        nc.gpsimd.affine_select(
            idn[:], junk[:, 0:128], pattern=[[-1, 128]],
            compare_op=mybir.AluOpType.is_equal, fill=0.0,
            channel_multiplier=1, base=0,
        )
        wb = const.tile([128, N], F32, tag="wb")
        nc.gpsimd.partition_broadcast(wb[:], w_row[:])

        wvt = [pwv.tile([128, D], F32R, tag=f"wv{c}", name=f"wv{c}") for c in range(KC)]
        for c in range(KC):
            eng = nc.sync if c % 2 == 0 else nc.scalar
            eng.dma_start(wvt[c][:], wv_d[ts(c, 128), :])
        bias = const.tile([128, KC], F32, tag="bias")
        nc.sync.dma_start(bias[:], bm_d.rearrange("(c p) -> p c", p=128))
        wmt = [pwm.tile([128, D], F16, tag=f"wm{c}", name=f"wm{c}") for c in range(KC)]
        for c in range(KC):
            eng = nc.sync if c % 2 == 0 else nc.scalar
            eng.dma_start(wmt[c][:], wm_d[ts(c, 128), :])

        qkt = [pqk.tile([128, N], F32R, tag=f"qk{m}", name=f"qk{m}")
               for m in range(2 * KC)]
        vt = [pv.tile([128, H * (HD + 1)], F16, tag=f"v{r}", name=f"v{r}")
              for r in range(NT)]
        ott = [pot.tile([128, N], F16, tag=f"ot{c}", name=f"ot{c}") for c in range(KC)]
        fin = [pfin.tile([128, N], F16, tag=f"fin{c}", name=f"fin{c}")
               for c in range(KC)]
        OFF = [0, 65, 130, 195, 512, 577, 642, 707]
        obufs: dict = {}
        if DEBUG:
            e_snap = const.tile([128, N], F32, tag="esnap")
            ob_snap = const.tile([128, 128], F32, tag="obsnap")

        def junk_mm(n):
            for _ in range(n):
                ps = psF.tile([128, 512], F32, tag="psF", name="psj")
                nc.tensor.matmul(ps[0:1, :], onescol[:, 0:1], junk[:],
                                 start=True, stop=True)

        junk_mm(JUNK0)

        ps_qk0 = psS.tile([128, N], F32, tag="psS", name="psq0")
        ps_qk6 = psS.tile([128, N], F32, tag="psS", name="psq6")
        for c in range(KC):
            for j in range(NC2):
                nc.tensor.matmul(ps_qk0[:, ts(j, 512)], wq_pre[:, ts(c, 128)],
                                 xtt[c][:, ts(j, 512)],
                                 start=(c == 0), stop=(c == KC - 1))
            for j in range(NC2):
                nc.tensor.matmul(ps_qk6[:, ts(j, 512)], wq_pre2[:, ts(c, 128)],
                                 xtt[c][:, ts(j, 512)],
                                 start=(c == 0), stop=(c == KC - 1))
            if c < KC - 1:
                junk_mm(3)
        nc.vector.tensor_mul(qkt[0][:], ps_qk0[:], wb[:])
        nc.vector.tensor_mul(qkt[KC][:], ps_qk6[:], wb[:])

        def veng(i):
            return nc.vector

        def gen_v():
            for r in range(NT):
                v3 = vt[r][:].rearrange("p (h e) -> p h e", e=HD + 1)
                nc.vector.tensor_copy(
                    v3[:, :, HD : HD + 1],
                    onescol[:].rearrange("p (h o) -> p h o", o=1),
                )
                for pi, (off, wd) in enumerate(((0, 512), (512, 256))):
                    ps = psF.tile([128, wd], F32, tag="psF", name="psv")
                    for c in range(KC):
                        nc.tensor.matmul(ps[:], xtt[c][:, ts(r, 128)],
                                         wvt[c][:, off : off + wd],
                                         start=(c == 0), stop=(c == KC - 1))
                        yield
                    h0, h1 = (0, 8) if pi == 0 else (8, 12)
                    nc.vector.tensor_scalar_mul(
                        v3[:, h0:h1, 0:HD],
                        ps[:].rearrange("p (h e) -> p h e", e=HD),
                        wcol[:, r : r + 1],
                    )

        def gen_qk(m):
            wq_m = load_wq(m, eng=nc.scalar if m % 2 else nc.sync)
            for j in range(NC2):
                ps = psF.tile([128, 512], F32, tag="psF", name="psq")
                for c in range(KC):
                    nc.tensor.matmul(ps[:], wq_m[:, ts(c, 128)],
                                     xtt[c][:, ts(j, 512)],
                                     start=(c == 0), stop=(c == KC - 1))
                    yield
                nc.vector.tensor_mul(
                    qkt[m][:, ts(j, 512)], ps[:], wb[:, ts(j, 512)]
                )

        def gen_final(ks, borrow=False):
            first = ks[0] == 0
            for cp in range(KC):
                for j in range(NC2):
                    pool = psS if (borrow and (cp * 2 + j) % 2) else psF
                    tag = "psS" if pool is psS else "psF"
                    ps = pool.tile([128, 512], F32, tag=tag, name="psy")
                    for i, k in enumerate(ks):
                        nc.tensor.matmul(ps[:], wmt[k][:, ts(cp, 128)],
                                         ott[k][:, ts(j, 512)],
                                         start=(i == 0), stop=(i == len(ks) - 1))
                        yield
                    dst = fin[cp][:, ts(j, 512)]
                    eng = veng(cp * 2 + j)
                    if first:
                        eng.tensor_scalar_add(dst, ps[:], bias[:, cp : cp + 1])
                    else:
                        eng.tensor_add(dst, ps[:], dst)

        fillq: deque = deque()

        def pump(units):
            while units >= 1.0 and fillq:
                try:
                    next(fillq[0])
                    units -= 1.0
                except StopIteration:
                    fillq.popleft()

        e_store: dict = {}

        def gen_S(h):
            c, half = h // 2, h % 2
            qt, kt = qkt[c], qkt[KC + c]
            qr = HD * half
            es = []
            for r in range(NT):
                ps = psS.tile([128, N], F32, tag="psS", name="ps")
                for j in range(NC2):
                    nc.tensor.matmul(ps[:, ts(j, 512)], kt[qr : qr + HD, ts(r, 128)],
                                     qt[qr : qr + HD, ts(j, 512)],
                                     start=True, stop=True)
                e = pe_.tile([128, N], F16, tag="e", name="e")
                nc.scalar.activation(e[:], ps[:], AF.Exp, scale=SCALE)
                if DEBUG and h == 0 and r == 0:
                    nc.vector.tensor_copy(e_snap[:], e[:])
                es.append(e)
                yield
            e_store[h] = es

        def gen_O(h):
            c, half = h // 2, h % 2
            es = e_store.pop(h)
            po = psO.tile([128, 1024], F32, tag="po", name="po")
            for qc in range(NT):
                for r in range(NT):
                    nc.tensor.matmul(po[:, OFF[qc] : OFF[qc] + HD + 1],
                                     es[r][:, ts(qc, 128)],
                                     vt[r][:, h * (HD + 1) : (h + 1) * (HD + 1)],
                                     start=(r == 0), stop=(r == NT - 1))
                yield
            rcs = prc.tile([128, NT], F32, tag="rcs", name="rcs")
            for half_b in range(2):
                nc.vector.tensor_copy(
                    rcs[:, 4 * half_b : 4 * half_b + 4].rearrange(
                        "p (g o) -> p g o", o=1),
                    po[:, 512 * half_b + HD : 512 * half_b + HD + 4 * (HD + 1)]
                    .rearrange("p (g e) -> p g e", e=HD + 1)[:, :, 0:1],
                )
            rc = prc.tile([128, NT], F32, tag="rc", name="rc")
            nc.vector.reciprocal(rc[:], rcs[:])
            for qc in range(NT):
                if half == 0:
                    obufs[(c, qc)] = pob.tile([128, 128], F16, tag=f"ob{qc}",
                                              name=f"ob{qc}")
                ob = obufs[(c, qc)]
                nc.vector.tensor_scalar_mul(
                    ob[:, HD * half : HD * half + HD],
                    po[:, OFF[qc] : OFF[qc] + HD],
                    rc[:, qc : qc + 1],
                )
                if DEBUG and h == 1 and qc == 0:
                    nc.vector.tensor_copy(ob_snap[:], ob[:])
                if half == 1:
                    pt = psF.tile([128, 128], F16, tag="psF", name="pt")
                    nc.tensor.transpose(pt[:], obufs.pop((c, qc))[:], idn[:])
                    if h == H - 1 and qc % 2 == 1:
                        nc.scalar.activation(ott[c][:, ts(qc, 128)], pt[:],
                                             AF.Identity)
                    else:
                        nc.vector.tensor_copy(ott[c][:, ts(qc, 128)], pt[:])
            yield

        v_gen = gen_v()
        fillq.append(v_gen)
        qk_gens = {}
        for m in (1, KC + 1, 2, KC + 2, 3, KC + 3, 4, KC + 4, 5, KC + 5):
            g = gen_qk(m)
            qk_gens[m] = g
            fillq.append(g)

        def drain(*targets):
            while any(g in fillq for g in targets):
                pump(1.0)

        prev_o = None
        for h in range(H):
            if h == 1:
                drain(v_gen)
            if h >= 2 and h % 2 == 0:
                drain(qk_gens[h // 2], qk_gens[KC + h // 2])
            if h == 8:
                fillq.append(gen_final((0, 1, 2, 3)))

            ratio = RATIOS[h]
            for _ in gen_S(h):
                if prev_o is not None:
                    next(prev_o, None)
                pump(ratio)
            if prev_o is not None:
                _run(prev_o)
            prev_o = gen_O(h)
        _run(prev_o)
        fillq.append(gen_final((4, 5), borrow=True))
        while fillq:
            pump(1e9)

        for cp in range(KC):
            eng = nc.sync if cp % 2 == 0 else nc.scalar
            eng.dma_start(y_d[ts(cp, 128), :], fin[cp][:])


def _build(repeat=1):
    key = ("nc", repeat)
    if key not in _CACHE:
        nc = bacc.Bacc("TRN2", target_bir_lowering=False, debug=False, num_devices=B)
        with tile.TileContext(nc) as tc:
            _emit(tc, repeat=repeat)
        nc.compile()
        _CACHE[key] = nc
    return _CACHE[key]


def kernel(x, weight, W_qkv, W_msa, b_msa):
    nc = _build()
    x = np.asarray(x, dtype=np.float32)
    weight = np.asarray(weight, dtype=np.float32)
    W_qkv = np.asarray(W_qkv, dtype=np.float32)
    wqk = np.ascontiguousarray(W_qkv[:, : 2 * D])
    wv = np.ascontiguousarray(W_qkv[:, 2 * D :])
    wm16 = np.asarray(W_msa, dtype=np.float16)
    in_maps = []
    for b in range(B):
        in_maps.append(
            {
                "xt": np.ascontiguousarray(x[b].T),
                "w": np.ascontiguousarray(weight[b : b + 1]),
                "wqk": wqk,
                "wv": wv,
                "wmsa": wm16,
                "bmsa": np.asarray(b_msa, dtype=np.float32),
            }
        )
    res = run_bass_kernel_spmd(nc, in_maps, list(range(B)))
    out = np.stack([res.results[b]["yt"].T for b in range(B)], axis=0)
    return np.ascontiguousarray(out.astype(np.float32))


# revision 41
# speedup vs baseline: 1.1454x; 1.0586x over previous
from collections import deque
from contextlib import ExitStack

import numpy as np

import concourse.bass as bass
import concourse.mybir as mybir
import concourse.tile as tile
from concourse import bacc
from concourse.bass import ts
from concourse.bass_utils import run_bass_kernel_spmd

B, N, D, H = 8, 1024, 768, 12
HD = D // H
SCALE = HD ** -0.5
KC = D // 128
NT = N // 128
NC2 = N // 512

F32 = mybir.dt.float32
F32R = mybir.dt.float32r
F16 = mybir.dt.float16
AF = mybir.ActivationFunctionType

RATIOS = {0: 6.0, 1: 2.5, 2: 1.8, 3: 1.8, 4: 1.8, 5: 1.8, 6: 1.8,
          7: 1.8, 8: 1.8, 9: 2.5, 10: 2.5, 11: 2.5}

_CACHE: dict = {}
DEBUG = False
JUNK0 = 12


def _run(gen):
    for _ in gen:
        pass


def _emit(tc, repeat=1):
    nc = tc.nc
    xt_d = nc.dram_tensor("xt", [D, N], F32R, kind="ExternalInput").ap()
    w_d = nc.dram_tensor("w", [1, N], F32, kind="ExternalInput").ap()
    wqk_d = nc.dram_tensor("wqk", [D, 2 * D], F32R, kind="ExternalInput").ap()
    wv_d = nc.dram_tensor("wv", [D, D], F32R, kind="ExternalInput").ap()
    wm_d = nc.dram_tensor("wmsa", [D, D], F16, kind="ExternalInput").ap()
    bm_d = nc.dram_tensor("bmsa", [D], F32, kind="ExternalInput").ap()
    y_d = nc.dram_tensor("yt", [D, N], F16, kind="ExternalOutput").ap()

    for _rep in range(repeat):
        _emit_body(tc, xt_d, w_d, wqk_d, wv_d, wm_d, bm_d, y_d)


def _emit_body(tc, xt_d, w_d, wqk_d, wv_d, wm_d, bm_d, y_d):
    nc = tc.nc
    with ExitStack() as s1:
        const = s1.enter_context(tc.tile_pool(name="const", bufs=1))
        pxt = s1.enter_context(tc.tile_pool(name="pxt", bufs=1))
        pwqs = s1.enter_context(tc.tile_pool(name="pwqs", bufs=2))
        pqk = s1.enter_context(tc.tile_pool(name="pqk", bufs=1))
        pwv = s1.enter_context(tc.tile_pool(name="pwv", bufs=1))
        pwm = s1.enter_context(tc.tile_pool(name="pwm", bufs=1))
        pv = s1.enter_context(tc.tile_pool(name="pv", bufs=1))
        pe_ = s1.enter_context(tc.tile_pool(name="pe", bufs=2 * NT + 1))
        pob = s1.enter_context(tc.tile_pool(name="pob", bufs=2))
        pot = s1.enter_context(tc.tile_pool(name="pot", bufs=1))
        prc = s1.enter_context(tc.tile_pool(name="prc", bufs=2))
        pfin = s1.enter_context(tc.tile_pool(name="pfin", bufs=1))
        pdn = s1.enter_context(tc.tile_pool(name="pdn", bufs=1))
        psS = s1.enter_context(tc.tile_pool(name="psS", bufs=2, space="PSUM"))
        psO = s1.enter_context(tc.tile_pool(name="psO", bufs=2, space="PSUM"))
        psF = s1.enter_context(tc.tile_pool(name="psF", bufs=2, space="PSUM"))

        wqk3 = wqk_d.rearrange("(c p) e -> p c e", p=128)

        def load_wq(m, eng=nc.sync):
            t = pwqs.tile([128, KC * 128], F32R, tag="wqs", name="wqs")
            eng.dma_start(
                t[:].rearrange("p (c e) -> p c e", e=128),
                wqk3[:, :, ts(m, 128)],
            )
            return t

        wq_pre = load_wq(0, eng=nc.scalar)
        wq_pre2 = load_wq(KC, eng=nc.scalar)
        xtt = [pxt.tile([128, N], F32R, tag=f"xt{c}", name=f"xt{c}") for c in range(KC)]
        for c in range(KC):
            eng = nc.sync if c % 2 == 0 else nc.scalar
            eng.dma_start(xtt[c][:], xt_d[ts(c, 128), :])
        w_row = pdn.tile([1, N], F32, tag="w_row", name="w_row")
        nc.sync.dma_start(w_row[:], w_d[:])
        wcol = const.tile([128, NT], F32, tag="wcol")
        nc.sync.dma_start(wcol[:], w_d[0, :].rearrange("(r p) -> p r", p=128))

        onescol = const.tile([128, H], F16, tag="onescol")
        nc.vector.memset(onescol[:], 1.0)
        junk = const.tile([128, 512], F16, tag="junk")
        nc.vector.memset(junk[:], 1.0)
        idn = const.tile([128, 128], F16, tag="idn")
        nc.gpsimd.affine_select(
            idn[:], junk[:, 0:128], pattern=[[-1, 128]],
            compare_op=mybir.AluOpType.is_equal, fill=0.0,
            channel_multiplier=1, base=0,
        )
        wb = const.tile([128, N], F32, tag="wb")
        nc.gpsimd.partition_broadcast(wb[:], w_row[:])

        wvt = [pwv.tile([128, D], F32R, tag=f"wv{c}", name=f"wv{c}") for c in range(KC)]
        for c in range(KC):
            eng = nc.sync if c % 2 == 0 else nc.scalar
            eng.dma_start(wvt[c][:], wv_d[ts(c, 128), :])
        bias = const.tile([128, KC], F32, tag="bias")
        nc.sync.dma_start(bias[:], bm_d.rearrange("(c p) -> p c", p=128))
        wmt = [pwm.tile([128, D], F16, tag=f"wm{c}", name=f"wm{c}") for c in range(KC)]
        for c in range(KC):
            eng = nc.sync if c % 2 == 0 else nc.scalar
            eng.dma_start(wmt[c][:], wm_d[ts(c, 128), :])

        qkt = [pqk.tile([128, N], F32R, tag=f"qk{m}", name=f"qk{m}")
               for m in range(2 * KC)]
        vt = [pv.tile([128, H * (HD + 1)], F16, tag=f"v{r}", name=f"v{r}")
              for r in range(NT)]
        ott = [pot.tile([128, N], F16, tag=f"ot{c}", name=f"ot{c}") for c in range(KC)]
        fin = [pfin.tile([128, N], F16, tag=f"fin{c}", name=f"fin{c}")
               for c in range(KC)]
        OFF = [0, 65, 130, 195]
        obufs: dict = {}
        if DEBUG:
            e_snap = const.tile([128, N], F32, tag="esnap")
            ob_snap = const.tile([128, 128], F32, tag="obsnap")

        def junk_mm(n):
            for _ in range(n):
                ps = psF.tile([128, 512], F32, tag="psF", name="psj")
                nc.tensor.matmul(ps[0:1, :], onescol[:, 0:1], junk[:],
                                 start=True, stop=True)

        junk_mm(JUNK0)

        ps_qk0 = psS.tile([128, N], F32, tag="psS", name="psq0")
        ps_qk6 = psS.tile([128, N], F32, tag="psS", name="psq6")
        for c in range(KC):
            for j in range(NC2):
                nc.tensor.matmul(ps_qk0[:, ts(j, 512)], wq_pre[:, ts(c, 128)],
                                 xtt[c][:, ts(j, 512)],
                                 start=(c == 0), stop=(c == KC - 1))
            for j in range(NC2):
                nc.tensor.matmul(ps_qk6[:, ts(j, 512)], wq_pre2[:, ts(c, 128)],
                                 xtt[c][:, ts(j, 512)],
                                 start=(c == 0), stop=(c == KC - 1))
            if c < KC - 1:
                junk_mm(3)
        nc.vector.tensor_mul(qkt[0][:], ps_qk0[:], wb[:])
        nc.vector.tensor_mul(qkt[KC][:], ps_qk6[:], wb[:])

        def veng(i):
            return nc.vector

        def gen_v():
            for r in range(NT):
                v3 = vt[r][:].rearrange("p (h e) -> p h e", e=HD + 1)
                nc.vector.tensor_copy(
                    v3[:, :, HD : HD + 1],
                    onescol[:].rearrange("p (h o) -> p h o", o=1),
                )
                for pi, (off, wd) in enumerate(((0, 512), (512, 256))):
                    ps = psF.tile([128, wd], F32, tag="psF", name="psv")
                    for c in range(KC):
                        nc.tensor.matmul(ps[:], xtt[c][:, ts(r, 128)],
                                         wvt[c][:, off : off + wd],
                                         start=(c == 0), stop=(c == KC - 1))
                        yield
                    h0, h1 = (0, 8) if pi == 0 else (8, 12)
                    nc.vector.tensor_scalar_mul(
                        v3[:, h0:h1, 0:HD],
                        ps[:].rearrange("p (h e) -> p h e", e=HD),
                        wcol[:, r : r + 1],
                    )

        def gen_qk(m):
            wq_m = load_wq(m, eng=nc.scalar if m % 2 else nc.sync)
            for j in range(NC2):
                ps = psF.tile([128, 512], F32, tag="psF", name="psq")
                for c in range(KC):
                    nc.tensor.matmul(ps[:], wq_m[:, ts(c, 128)],
                                     xtt[c][:, ts(j, 512)],
                                     start=(c == 0), stop=(c == KC - 1))
                    yield
                nc.vector.tensor_mul(
                    qkt[m][:, ts(j, 512)], ps[:], wb[:, ts(j, 512)]
                )

        def gen_final(ks, borrow=False):
            first = ks[0] == 0
            for cp in range(KC):
                for j in range(NC2):
                    pool = psS if (borrow and (cp * 2 + j) % 2) else psF
                    tag = "psS" if pool is psS else "psF"
                    ps = pool.tile([128, 512], F32, tag=tag, name="psy")
                    for i, k in enumerate(ks):
                        nc.tensor.matmul(ps[:], wmt[k][:, ts(cp, 128)],
                                         ott[k][:, ts(j, 512)],
                                         start=(i == 0), stop=(i == len(ks) - 1))
                        yield
                    dst = fin[cp][:, ts(j, 512)]
                    eng = veng(cp * 2 + j)
                    if first:
                        eng.tensor_scalar_add(dst, ps[:], bias[:, cp : cp + 1])
                    else:
                        eng.tensor_add(dst, ps[:], dst)

        fillq: deque = deque()

        def pump(units):
            while units >= 1.0 and fillq:
                try:
                    next(fillq[0])
                    units -= 1.0
                except StopIteration:
                    fillq.popleft()

        e_store: dict = {}

        def gen_S(h):
            c, half = h // 2, h % 2
            qt, kt = qkt[c], qkt[KC + c]
            qr = HD * half
            es = []
            for r in range(NT):
                ps = psS.tile([128, N], F32, tag="psS", name="ps")
                for j in range(NC2):
                    nc.tensor.matmul(ps[:, ts(j, 512)], kt[qr : qr + HD, ts(r, 128)],
                                     qt[qr : qr + HD, ts(j, 512)],
                                     start=True, stop=True)
                e = pe_.tile([128, N], F16, tag="e", name="e")
                nc.scalar.activation(e[:], ps[:], AF.Exp, scale=SCALE)
                if DEBUG and h == 0 and r == 0:
                    nc.vector.tensor_copy(e_snap[:], e[:])
                es.append(e)
                yield
            e_store[h] = es

        def gen_O(h):
            c, half = h // 2, h % 2
            es = e_store.pop(h)
            for th in range(2):
                po = psO.tile([128, 512], F32, tag="po", name="po")
                for qi in range(4):
                    qc = 4 * th + qi
                    for r in range(NT):
                        nc.tensor.matmul(po[:, OFF[qi] : OFF[qi] + HD + 1],
                                         es[r][:, ts(qc, 128)],
                                         vt[r][:, h * (HD + 1) : (h + 1) * (HD + 1)],
                                         start=(r == 0), stop=(r == NT - 1))
                    yield
                rcs = prc.tile([128, 4], F32, tag="rcs", name="rcs")
                nc.vector.tensor_copy(
                    rcs[:].rearrange("p (g o) -> p g o", o=1),
                    po[:, HD : HD + 4 * (HD + 1)]
                    .rearrange("p (g e) -> p g e", e=HD + 1)[:, :, 0:1],
                )
                rc = prc.tile([128, 4], F32, tag="rc", name="rc")
                nc.vector.reciprocal(rc[:], rcs[:])
                for qi in range(4):
                    qc = 4 * th + qi
                    if half == 0:
                        obufs[(c, qc)] = pob.tile([128, 128], F16, tag=f"ob{qc}",
                                                  name=f"ob{qc}")
                    ob = obufs[(c, qc)]
                    nc.vector.tensor_scalar_mul(
                        ob[:, HD * half : HD * half + HD],
                        po[:, OFF[qi] : OFF[qi] + HD],
                        rc[:, qi : qi + 1],
                    )
                    if DEBUG and h == 1 and qc == 0:
                        nc.vector.tensor_copy(ob_snap[:], ob[:])
                    if half == 1:
                        pt = psF.tile([128, 128], F16, tag="psF", name="pt")
                        nc.tensor.transpose(pt[:], obufs.pop((c, qc))[:], idn[:])
                        if h == H - 1 and qc % 2 == 1:
                            nc.scalar.activation(ott[c][:, ts(qc, 128)], pt[:],
                                                 AF.Identity)
                        else:
                            nc.vector.tensor_copy(ott[c][:, ts(qc, 128)], pt[:])
                yield

        v_gen = gen_v()
        fillq.append(v_gen)
        qk_gens = {}
        for m in (1, KC + 1, 2, KC + 2, 3, KC + 3, 4, KC + 4, 5, KC + 5):
            g = gen_qk(m)
            qk_gens[m] = g
            fillq.append(g)

        def drain(*targets):
            while any(g in fillq for g in targets):
                pump(1.0)

        prev_o = None
        for h in range(H):
            if h == 1:
                drain(v_gen)
            if h >= 2 and h % 2 == 0:
                drain(qk_gens[h // 2], qk_gens[KC + h // 2])
            if h == 8:
                fillq.append(gen_final((0, 1, 2, 3)))

            ratio = RATIOS[h]
            for _ in gen_S(h):
                if prev_o is not None:
                    next(prev_o, None)
                pump(ratio)
            if prev_o is not None:
                _run(prev_o)
            prev_o = gen_O(h)
        _run(prev_o)
        fillq.append(gen_final((4, 5), borrow=True))
        while fillq:
            pump(1e9)

        for cp in range(KC):
            eng = nc.sync if cp % 2 == 0 else nc.scalar
            eng.dma_start(y_d[ts(cp, 128), :], fin[cp][:])


def _build(repeat=1):
    key = ("nc", repeat)
    if key not in _CACHE:
        nc = bacc.Bacc("TRN2", target_bir_lowering=False, debug=False, num_devices=B)
        with tile.TileContext(nc) as tc:
            _emit(tc, repeat=repeat)
        nc.compile()
        _CACHE[key] = nc
    return _CACHE[key]


def kernel(x, weight, W_qkv, W_msa, b_msa):
    nc = _build()
    x = np.asarray(x, dtype=np.float32)
    weight = np.asarray(weight, dtype=np.float32)
    W_qkv = np.asarray(W_qkv, dtype=np.float32)
    wqk = np.ascontiguousarray(W_qkv[:, : 2 * D])
    wv = np.ascontiguousarray(W_qkv[:, 2 * D :])
    wm16 = np.asarray(W_msa, dtype=np.float16)
    in_maps = []
    for b in range(B):
        in_maps.append(
            {
                "xt": np.ascontiguousarray(x[b].T),
                "w": np.ascontiguousarray(weight[b : b + 1]),
                "wqk": wqk,
                "wv": wv,
                "wmsa": wm16,
                "bmsa": np.asarray(b_msa, dtype=np.float32),
            }
        )
    res = run_bass_kernel_spmd(nc, in_maps, list(range(B)))
    out = np.stack([res.results[b]["yt"].T for b in range(B)], axis=0)
    return np.ascontiguousarray(out.astype(np.float32))


# revision 42
# speedup vs baseline: 1.1508x; 1.0048x over previous
from collections import deque
from contextlib import ExitStack

import numpy as np

import concourse.bass as bass
import concourse.mybir as mybir
import concourse.tile as tile
from concourse import bacc
from concourse.bass import ts
from concourse.bass_utils import run_bass_kernel_spmd

B, N, D, H = 8, 1024, 768, 12
HD = D // H
SCALE = HD ** -0.5
KC = D // 128
NT = N // 128
NC2 = N // 512

F32 = mybir.dt.float32
F32R = mybir.dt.float32r
F16 = mybir.dt.float16
AF = mybir.ActivationFunctionType

RATIOS = {0: 6.0, 1: 2.5, 2: 1.8, 3: 1.8, 4: 1.8, 5: 1.8, 6: 1.8,
          7: 1.8, 8: 1.8, 9: 2.5, 10: 2.5, 11: 2.5}
RW = [1.6, 1.6, 1.4, 1.2, 1.0, 0.8, 0.2, 0.2]

_CACHE: dict = {}
DEBUG = False
JUNK0 = 12


def _run(gen):
    for _ in gen:
        pass


def _emit(tc, repeat=1):
    nc = tc.nc
    xt_d = nc.dram_tensor("xt", [D, N], F32R, kind="ExternalInput").ap()
    w_d = nc.dram_tensor("w", [1, N], F32, kind="ExternalInput").ap()
    wqk_d = nc.dram_tensor("wqk", [D, 2 * D], F32R, kind="ExternalInput").ap()
    wv_d = nc.dram_tensor("wv", [D, D], F32R, kind="ExternalInput").ap()
    wm_d = nc.dram_tensor("wmsa", [D, D], F16, kind="ExternalInput").ap()
    bm_d = nc.dram_tensor("bmsa", [D], F32, kind="ExternalInput").ap()
    y_d = nc.dram_tensor("yt", [D, N], F16, kind="ExternalOutput").ap()

    for _rep in range(repeat):
        _emit_body(tc, xt_d, w_d, wqk_d, wv_d, wm_d, bm_d, y_d)


def _emit_body(tc, xt_d, w_d, wqk_d, wv_d, wm_d, bm_d, y_d):
    nc = tc.nc
    with ExitStack() as s1:
        const = s1.enter_context(tc.tile_pool(name="const", bufs=1))
        pxt = s1.enter_context(tc.tile_pool(name="pxt", bufs=1))
        pwqs = s1.enter_context(tc.tile_pool(name="pwqs", bufs=2))
        pqk = s1.enter_context(tc.tile_pool(name="pqk", bufs=1))
        pwv = s1.enter_context(tc.tile_pool(name="pwv", bufs=1))
        pwm = s1.enter_context(tc.tile_pool(name="pwm", bufs=1))
        pv = s1.enter_context(tc.tile_pool(name="pv", bufs=1))
        pe_ = s1.enter_context(tc.tile_pool(name="pe", bufs=2 * NT + 1))
        pob = s1.enter_context(tc.tile_pool(name="pob", bufs=2))
        pot = s1.enter_context(tc.tile_pool(name="pot", bufs=1))
        prc = s1.enter_context(tc.tile_pool(name="prc", bufs=2))
        pfin = s1.enter_context(tc.tile_pool(name="pfin", bufs=1))
        pdn = s1.enter_context(tc.tile_pool(name="pdn", bufs=1))
        psS = s1.enter_context(tc.tile_pool(name="psS", bufs=2, space="PSUM"))
        psO = s1.enter_context(tc.tile_pool(name="psO", bufs=2, space="PSUM"))
        psF = s1.enter_context(tc.tile_pool(name="psF", bufs=2, space="PSUM"))

        wqk3 = wqk_d.rearrange("(c p) e -> p c e", p=128)

        def load_wq(m, eng=nc.sync):
            t = pwqs.tile([128, KC * 128], F32R, tag="wqs", name="wqs")
            eng.dma_start(
                t[:].rearrange("p (c e) -> p c e", e=128),
                wqk3[:, :, ts(m, 128)],
            )
            return t

        wq_pre = load_wq(0, eng=nc.scalar)
        wq_pre2 = load_wq(KC, eng=nc.scalar)
        xtt = [pxt.tile([128, N], F32R, tag=f"xt{c}", name=f"xt{c}") for c in range(KC)]
        for c in range(KC):
            eng = nc.sync if c % 2 == 0 else nc.scalar
            eng.dma_start(xtt[c][:], xt_d[ts(c, 128), :])
        w_row = pdn.tile([1, N], F32, tag="w_row", name="w_row")
        nc.sync.dma_start(w_row[:], w_d[:])
        wcol = const.tile([128, NT], F32, tag="wcol")
        nc.sync.dma_start(wcol[:], w_d[0, :].rearrange("(r p) -> p r", p=128))

        onescol = const.tile([128, H], F16, tag="onescol")
        nc.vector.memset(onescol[:], 1.0)
        junk = const.tile([128, 512], F16, tag="junk")
        nc.vector.memset(junk[:], 1.0)
        idn = const.tile([128, 128], F16, tag="idn")
        nc.gpsimd.affine_select(
            idn[:], junk[:, 0:128], pattern=[[-1, 128]],
            compare_op=mybir.AluOpType.is_equal, fill=0.0,
            channel_multiplier=1, base=0,
        )
        wb = const.tile([128, N], F32, tag="wb")
        nc.gpsimd.partition_broadcast(wb[:], w_row[:])

        wvt = [pwv.tile([128, D], F32R, tag=f"wv{c}", name=f"wv{c}") for c in range(KC)]
        for c in range(KC):
            eng = nc.sync if c % 2 == 0 else nc.scalar
            eng.dma_start(wvt[c][:], wv_d[ts(c, 128), :])
        bias = const.tile([128, KC], F32, tag="bias")
        nc.sync.dma_start(bias[:], bm_d.rearrange("(c p) -> p c", p=128))
        wmt = [pwm.tile([128, D], F16, tag=f"wm{c}", name=f"wm{c}") for c in range(KC)]
        for c in range(KC):
            eng = nc.sync if c % 2 == 0 else nc.scalar
            eng.dma_start(wmt[c][:], wm_d[ts(c, 128), :])

        qkt = [pqk.tile([128, N], F32R, tag=f"qk{m}", name=f"qk{m}")
               for m in range(2 * KC)]
        vt = [pv.tile([128, H * (HD + 1)], F16, tag=f"v{r}", name=f"v{r}")
              for r in range(NT)]
        ott = [pot.tile([128, N], F16, tag=f"ot{c}", name=f"ot{c}") for c in range(KC)]
        fin = [pfin.tile([128, N], F16, tag=f"fin{c}", name=f"fin{c}")
               for c in range(KC)]
        OFF = [0, 65, 130, 195]
        obufs: dict = {}
        if DEBUG:
            e_snap = const.tile([128, N], F32, tag="esnap")
            ob_snap = const.tile([128, 128], F32, tag="obsnap")

        def junk_mm(n):
            for _ in range(n):
                ps = psF.tile([128, 512], F32, tag="psF", name="psj")
                nc.tensor.matmul(ps[0:1, :], onescol[:, 0:1], junk[:],
                                 start=True, stop=True)

        junk_mm(JUNK0)

        ps_qk0 = psS.tile([128, N], F32, tag="psS", name="psq0")
        ps_qk6 = psS.tile([128, N], F32, tag="psS", name="psq6")
        for c in range(KC):
            for j in range(NC2):
                nc.tensor.matmul(ps_qk0[:, ts(j, 512)], wq_pre[:, ts(c, 128)],
                                 xtt[c][:, ts(j, 512)],
                                 start=(c == 0), stop=(c == KC - 1))
            for j in range(NC2):
                nc.tensor.matmul(ps_qk6[:, ts(j, 512)], wq_pre2[:, ts(c, 128)],
                                 xtt[c][:, ts(j, 512)],
                                 start=(c == 0), stop=(c == KC - 1))
            if c < KC - 1:
                junk_mm(3)
        nc.vector.tensor_mul(qkt[0][:], ps_qk0[:], wb[:])
        nc.vector.tensor_mul(qkt[KC][:], ps_qk6[:], wb[:])

        def veng(i):
            return nc.vector

        def gen_v():
            for r in range(NT):
                v3 = vt[r][:].rearrange("p (h e) -> p h e", e=HD + 1)
                nc.vector.tensor_copy(
                    v3[:, :, HD : HD + 1],
                    onescol[:].rearrange("p (h o) -> p h o", o=1),
                )
                for pi, (off, wd) in enumerate(((0, 512), (512, 256))):
                    ps = psF.tile([128, wd], F32, tag="psF", name="psv")
                    for c in range(KC):
                        nc.tensor.matmul(ps[:], xtt[c][:, ts(r, 128)],
                                         wvt[c][:, off : off + wd],
                                         start=(c == 0), stop=(c == KC - 1))
                        yield
                    h0, h1 = (0, 8) if pi == 0 else (8, 12)
                    nc.vector.tensor_scalar_mul(
                        v3[:, h0:h1, 0:HD],
                        ps[:].rearrange("p (h e) -> p h e", e=HD),
                        wcol[:, r : r + 1],
                    )

        def gen_qk(m):
            wq_m = load_wq(m, eng=nc.scalar if m % 2 else nc.sync)
            for j in range(NC2):
                ps = psF.tile([128, 512], F32, tag="psF", name="psq")
                for c in range(KC):
                    nc.tensor.matmul(ps[:], wq_m[:, ts(c, 128)],
                                     xtt[c][:, ts(j, 512)],
                                     start=(c == 0), stop=(c == KC - 1))
                    yield
                nc.vector.tensor_mul(
                    qkt[m][:, ts(j, 512)], ps[:], wb[:, ts(j, 512)]
                )

        def gen_final(ks, borrow=False):
            first = ks[0] == 0
            for cp in range(KC):
                for j in range(NC2):
                    pool = psS if (borrow and (cp * 2 + j) % 2) else psF
                    tag = "psS" if pool is psS else "psF"
                    ps = pool.tile([128, 512], F32, tag=tag, name="psy")
                    for i, k in enumerate(ks):
                        nc.tensor.matmul(ps[:], wmt[k][:, ts(cp, 128)],
                                         ott[k][:, ts(j, 512)],
                                         start=(i == 0), stop=(i == len(ks) - 1))
                        yield
                    dst = fin[cp][:, ts(j, 512)]
                    eng = veng(cp * 2 + j)
                    if first:
                        eng.tensor_scalar_add(dst, ps[:], bias[:, cp : cp + 1])
                    else:
                        eng.tensor_add(dst, ps[:], dst)

        fillq: deque = deque()

        def pump(units):
            while units >= 1.0 and fillq:
                try:
                    next(fillq[0])
                    units -= 1.0
                except StopIteration:
                    fillq.popleft()

        e_store: dict = {}

        def gen_S(h):
            c, half = h // 2, h % 2
            qt, kt = qkt[c], qkt[KC + c]
            qr = HD * half
            es = []
            for r in range(NT):
                ps = psS.tile([128, N], F32, tag="psS", name="ps")
                for j in range(NC2):
                    nc.tensor.matmul(ps[:, ts(j, 512)], kt[qr : qr + HD, ts(r, 128)],
                                     qt[qr : qr + HD, ts(j, 512)],
                                     start=True, stop=True)
                e = pe_.tile([128, N], F16, tag="e", name="e")
                nc.scalar.activation(e[:], ps[:], AF.Exp, scale=SCALE)
                if DEBUG and h == 0 and r == 0:
                    nc.vector.tensor_copy(e_snap[:], e[:])
                es.append(e)
                yield
            e_store[h] = es

        def gen_O(h):
            c, half = h // 2, h % 2
            es = e_store.pop(h)
            for th in range(2):
                po = psO.tile([128, 512], F32, tag="po", name="po")
                for qi in range(4):
                    qc = 4 * th + qi
                    for r in range(NT):
                        nc.tensor.matmul(po[:, OFF[qi] : OFF[qi] + HD + 1],
                                         es[r][:, ts(qc, 128)],
                                         vt[r][:, h * (HD + 1) : (h + 1) * (HD + 1)],
                                         start=(r == 0), stop=(r == NT - 1))
                    yield
                rcs = prc.tile([128, 4], F32, tag="rcs", name="rcs")
                nc.vector.tensor_copy(
                    rcs[:].rearrange("p (g o) -> p g o", o=1),
                    po[:, HD : HD + 4 * (HD + 1)]
                    .rearrange("p (g e) -> p g e", e=HD + 1)[:, :, 0:1],
                )
                rc = prc.tile([128, 4], F32, tag="rc", name="rc")
                nc.vector.reciprocal(rc[:], rcs[:])
                for qi in range(4):
                    qc = 4 * th + qi
                    if half == 0:
                        obufs[(c, qc)] = pob.tile([128, 128], F16, tag=f"ob{qc}",
                                                  name=f"ob{qc}")
                    ob = obufs[(c, qc)]
                    nc.vector.tensor_scalar_mul(
                        ob[:, HD * half : HD * half + HD],
                        po[:, OFF[qi] : OFF[qi] + HD],
                        rc[:, qi : qi + 1],
                    )
                    if DEBUG and h == 1 and qc == 0:
                        nc.vector.tensor_copy(ob_snap[:], ob[:])
                    if half == 1:
                        pt = psF.tile([128, 128], F16, tag="psF", name="pt")
                        nc.tensor.transpose(pt[:], obufs.pop((c, qc))[:], idn[:])
                        if h == H - 1 and qc % 2 == 1:
                            nc.scalar.activation(ott[c][:, ts(qc, 128)], pt[:],
                                                 AF.Identity)
                        else:
                            nc.vector.tensor_copy(ott[c][:, ts(qc, 128)], pt[:])
                yield

        v_gen = gen_v()
        fillq.append(v_gen)
        qk_gens = {}
        for m in (1, KC + 1, 2, KC + 2, 3, KC + 3, 4, KC + 4, 5, KC + 5):
            g = gen_qk(m)
            qk_gens[m] = g
            fillq.append(g)

        def drain(*targets):
            while any(g in fillq for g in targets):
                pump(1.0)

        prev_o = None
        for h in range(H):
            if h == 1:
                drain(v_gen)
            if h >= 2 and h % 2 == 0:
                drain(qk_gens[h // 2], qk_gens[KC + h // 2])
            if h == 8:
                fillq.append(gen_final((0, 1, 2, 3)))

            ratio = RATIOS[h]
            for i, _ in enumerate(gen_S(h)):
                if prev_o is not None:
                    next(prev_o, None)
                pump(ratio * RW[i])
            if prev_o is not None:
                _run(prev_o)
            prev_o = gen_O(h)
        _run(prev_o)
        fillq.append(gen_final((4, 5), borrow=True))
        while fillq:
            pump(1e9)

        for cp in range(KC):
            eng = nc.sync if cp % 2 == 0 else nc.scalar
            eng.dma_start(y_d[ts(cp, 128), :], fin[cp][:])


def _build(repeat=1):
    key = ("nc", repeat)
    if key not in _CACHE:
        nc = bacc.Bacc("TRN2", target_bir_lowering=False, debug=False, num_devices=B)
        with tile.TileContext(nc) as tc:
            _emit(tc, repeat=repeat)
        nc.compile()
        _CACHE[key] = nc
    return _CACHE[key]


def kernel(x, weight, W_qkv, W_msa, b_msa):
    nc = _build()
    x = np.asarray(x, dtype=np.float32)
    weight = np.asarray(weight, dtype=np.float32)
    W_qkv = np.asarray(W_qkv, dtype=np.float32)
    wqk = np.ascontiguousarray(W_qkv[:, : 2 * D])
    wv = np.ascontiguousarray(W_qkv[:, 2 * D :])
    wm16 = np.asarray(W_msa, dtype=np.float16)
    in_maps = []
    for b in range(B):
        in_maps.append(
            {
                "xt": np.ascontiguousarray(x[b].T),
                "w": np.ascontiguousarray(weight[b : b + 1]),
                "wqk": wqk,
                "wv": wv,
                "wmsa": wm16,
                "bmsa": np.asarray(b_msa, dtype=np.float32),
            }
        )
    res = run_bass_kernel_spmd(nc, in_maps, list(range(B)))
    out = np.stack([res.results[b]["yt"].T for b in range(B)], axis=0)
    return np.ascontiguousarray(out.astype(np.float32))
